# revision 9
# baseline (speedup 1.0000x reference)
# Fused conv3x3(same) + bias + tanh + x2 + stride-4 subsample, data-parallel
# over 8 NeuronCores.
#
# Math: out[b,oc,y,x] = 2*tanh(sum_{ic,ky,kx} w[oc,ic,ky,kx]*x[b,ic,4y+ky-1,4x+kx-1] + bias[oc])
# computed in fp16 like the reference. Since the spatial stride (4) exceeds the
# kernel size (3), every output pixel reads a disjoint 3x3x8 input patch, so the
# conv lowers exactly to a [72 -> 64] GEMM over 64*64 pixels per image. The host
# does the im2col rearrangement (pure data movement, fp16 cast is identical to
# the reference's .astype(float16)); each core runs the GEMM + bias + tanh for
# 4 of the 32 images. The trailing *2 and fp32 cast are exact in either order,
# so they are applied on the host after the fp16 tanh.
#
# Device kernel is hand-scheduled raw bacc. The core's 16384 pixels are cut
# into 32 chunks of 512; chunk g -> one matmul into PSUM partitions (g%2)*64..
# of bank (g//2)%8. Chunks are grouped into 9 pipeline grains
# (2,2,4,4,4,4,4,4,4 chunks): per grain one input DMA -> matmuls -> one
# 128-partition ACT (tanh + per-partition bias via the ACT bias operand - no
# bias row in the contraction) -> one output DMA. Small head grains start the
# serial scalar ACT chain ~2us earlier.
#
# Scheduling notes (from perfetto traces):
# - HBM reads cap at ~16 GB/s per SDMA engine (latency-bound, size-flat), and
#   a DMA's partition count must be a multiple of 16 to engage all 16 engines
#   (descriptor quantum = smallest divisor >= P/16; 72 rows -> 12 engines).
#   Hence the 72 contraction rows are padded to 80.
# - ALL DMAs ride the sync HWDGE ring, inputs in grain order: per-ring FIFO
#   completes early grains first. The scalar HWDGE ring stalls DMAs ~4us and
#   splitting one stream across rings round-robins the engines between
#   streams, delaying early completions - both measured. DRAM rows are
#   strided (32KB apart) so concurrent descriptors hit different HBM banks.
# - Scalar ACT chain (tanh, (N+352)/1.2 ns) is the serial tail; a dummy
#   1-col ACT at the scalar queue head hoists the ~1.3us ACT_TABLE_LOAD into
#   the (fixed ~7us) framework preamble.
# - The PE clock gate opens only after ~5us of sustained matmul activity
#   (cold MMs run at 1.2GHz, warm 2.4GHz): a warmup burst bridges
#   preamble -> first real matmul, fillers bridge later input waits.
import sys

import numpy as np

try:
    import concourse.bass as bass  # noqa: F401
except ImportError:
    sys.path.insert(0, "/opt/trn_rl_repo")

import concourse.bass as bass  # noqa: F401
import concourse.bacc as bacc
import concourse.mybir as mybir
from concourse.bass_utils import run_bass_kernel_spmd

N_CORES = 8
B_FULL = 32
B_CORE = B_FULL // N_CORES  # 4 images per core
C_IN = 8
KH = KW = 3
K = C_IN * KH * KW  # 72 real contraction rows
KP = 80  # padded to a multiple of 16 so input DMAs engage all 16 SDMA engines
OC = 64
OH = OW = 64
NPIX = OH * OW  # 4096
NCHUNK = B_CORE * NPIX // 512  # 32 pixel chunks of 512
GRAINS = [2, 2, 4, 4, 4, 4, 4, 4, 4]  # chunks per pipeline grain (sum 32)
N_WARM = 15
F16 = mybir.dt.float16
F32 = mybir.dt.float32

_PROGRAM = None


def build_program():
    from contextlib import ExitStack

    assert sum(GRAINS) == NCHUNK
    starts = [sum(GRAINS[:j]) for j in range(len(GRAINS) + 1)]
    nc = bacc.Bacc("TRN2")
    xp = nc.dram_tensor("xp", [KP, NCHUNK * 512], F16, kind="ExternalInput")
    w = nc.dram_tensor("w", [KP, OC], F16, kind="ExternalInput")
    b = nc.dram_tensor("b", [2 * OC, 1], F32, kind="ExternalInput")
    y = nc.dram_tensor("y", [2 * OC, NCHUNK * 256], F16, kind="ExternalOutput")

    with ExitStack() as stack:
        w_tile = stack.enter_context(nc.sbuf_tensor([KP, OC], F16))
        bias_tile = stack.enter_context(nc.sbuf_tensor([2 * OC, 1], F32))
        scratch = stack.enter_context(nc.sbuf_tensor([1, 2], F16))
        x_flat = stack.enter_context(nc.sbuf_tensor([KP, NCHUNK * 512], F16))
        a_flat = stack.enter_context(nc.sbuf_tensor([2 * OC, NCHUNK * 256], F16))
        warm = stack.enter_context(nc.sbuf_tensor([2 * OC, 2 * OC], F16))
        # 8 banks of [128, 512]; chunk g -> partitions (g%2)*64.., bank (g//2)%8
        ps = stack.enter_context(nc.psum_tensor([2 * OC, 8, 512], F32))
        # Per-grain input semaphores: concurrent DMAs complete out of order,
        # so one counting sem can't tell which transfer landed.
        sx = [stack.enter_context(nc.semaphore(f"s_x{j}")) for j in range(len(GRAINS))]
        s_w = stack.enter_context(nc.semaphore("s_w"))
        s_b = stack.enter_context(nc.semaphore("s_b"))
        s_warm = stack.enter_context(nc.semaphore("s_warm"))
        s_mm = stack.enter_context(nc.semaphore("s_mm"))
        s_act = stack.enter_context(nc.semaphore("s_act"))
        s_y = stack.enter_context(nc.semaphore("s_y"))
        block = stack.enter_context(nc.Block())

        @block.gpsimd
        def _(gpsimd):
            gpsimd.memset(warm[:], 0.0).then_inc(s_warm, 1)

        @block.sync
        def _(sync):
            # single ring, grain order; w/b interleaved early (tiny drains,
            # engines are drain-limited so their issue slots don't delay x)
            sync.dma_start(out=w_tile[:], in_=w[:]).then_inc(s_w, 16)
            for j in range(len(GRAINS)):
                a, e = starts[j] * 512, starts[j + 1] * 512
                sync.dma_start(out=x_flat[:, a:e], in_=xp[:, a:e]).then_inc(sx[j], 16)
                if j == 1:
                    sync.dma_start(out=bias_tile[:], in_=b[:]).then_inc(s_b, 16)
            # output stores, paced by the ACT chain (2KB/partition runs max)
            for j in range(len(GRAINS)):
                a, e = starts[j] * 256, starts[j + 1] * 256
                sync.wait_ge(s_act, j + 1)
                sync.dma_start(out=y[:, a:e], in_=a_flat[:, a:e]).then_inc(s_y, 16)
            sync.wait_ge(s_y, 16 * len(GRAINS))

        @block.scalar
        def _(scalar):
            # dummy 1-col activation: hoists the ACT_TABLE_LOAD to the queue
            # head so it overlaps the framework preamble instead of delaying
            # the first real ACT. Reads/writes a scratch tile (garbage ok).
            nc.scalar.activation(
                scratch[:, 0:1], scratch[:, 1:2], mybir.ActivationFunctionType.Tanh
            )
            for j in range(len(GRAINS)):
                scalar.wait_ge(s_mm, j + 1)
                if j == 0:
                    scalar.wait_ge(s_b, 16)
                b0, b1 = (starts[j] // 2) % 8, ((starts[j + 1] - 1) // 2) % 8 + 1
                nc.scalar.activation(
                    a_flat[:, starts[j] * 256 : starts[j + 1] * 256],
                    ps[:, b0:b1, :].rearrange("p b c -> p (b c)"),
                    mybir.ActivationFunctionType.Tanh,
                    bias=bias_tile[:, 0:1],
                ).then_inc(s_act, 1)

        @block.tensor
        def _(tensor):
            # keep the PE busy from preamble-exit until grain-0 input lands so
            # the HAM clock gate ramp starts early; results land in bank 7
            # which chunk 14 later overwrites (start=True)
            tensor.wait_ge(s_warm, 1)
            for _ in range(N_WARM):
                nc.tensor.matmul(
                    ps[:OC, 7, :128], warm[:, :OC], warm[:], start=True, stop=True
                )
            for j in range(len(GRAINS)):
                if j == 0:
                    tensor.wait_ge(s_w, 16)
                if starts[j] >= 16:
                    # psum banks reused; wait until the ACT that read the
                    # previous occupants (chunks starts[j]-16..) is done.
                    # Taken BEFORE the input wait so the fillers below may
                    # touch this grain's banks.
                    tensor.wait_ge(s_act, j - 3)
                    # fillers: keep the PE busy across the input wait so the
                    # clock gate stays open; they write this grain's own
                    # first bank, which the real start=True matmuls overwrite
                    for _ in range(2):
                        nc.tensor.matmul(
                            ps[:OC, (starts[j] // 2) % 8, :128],
                            warm[:, :OC],
                            warm[:],
                            start=True,
                            stop=True,
                        )
                tensor.wait_ge(sx[j], 16)
                last = None
                for g in range(starts[j], starts[j + 1]):
                    last = nc.tensor.matmul(
                        ps[(g % 2) * OC : (g % 2 + 1) * OC, (g // 2) % 8, :],
                        w_tile[:],
                        x_flat[:, g * 512 : (g + 1) * 512],
                        start=True,
                        stop=True,
                    )
                last.then_inc(s_mm, 1)

    nc.finalize()
    return nc


def _get_program():
    global _PROGRAM
    if _PROGRAM is None:
        _PROGRAM = build_program()
    return _PROGRAM


def _im2col(x: np.ndarray) -> np.ndarray:
    """[B,8,256,256] fp32 -> [B,80,4096] fp16 patches, p=(ky*3+kx)*8+ic,
    rows 72..79 zero (pad so input DMAs engage all 16 SDMA engines)."""
    B, C, H, W = x.shape
    xh = x.astype(np.float16)
    xpad = np.zeros((B, C, H + 2, W + 2), np.float16)
    xpad[:, :, 1 : H + 1, 1 : W + 1] = xh
    s = xpad.strides
    # windows[b,c,ky,kx,y,x] = xpad[b,c,4y+ky,4x+kx] = x[b,c,4y+ky-1,4x+kx-1]
    win = np.lib.stride_tricks.as_strided(
        xpad,
        shape=(B, C, KH, KW, OH, OW),
        strides=(s[0], s[1], s[2], s[3], 4 * s[2], 4 * s[3]),
    )
    out = np.zeros((B, KP, NPIX), np.float16)
    np.copyto(
        out[:, :K].reshape(B, KH, KW, C, OH, OW), win.transpose(0, 2, 3, 1, 4, 5)
    )
    return out


def run_sharded(x, weight, bias, **spmd_kwargs):
    """Returns (output, BassKernelResults). spmd_kwargs e.g. trace=True."""
    patches = _im2col(x)  # [32, 80, 4096] f16 (rows 72-79 zero)
    w_mat = np.zeros((KP, OC), np.float16)
    w_mat[:K] = weight.transpose(2, 3, 1, 0).reshape(K, OC).astype(np.float16)
    b_vec = np.ascontiguousarray(
        np.tile(bias.astype(np.float32).reshape(OC), 2)[:, None]
    )

    in_maps = [
        {
            # [80, 16384]: 4 images side by side, pixel-major per row;
            # 32KB row stride spreads concurrent descriptors across HBM banks
            "xp": np.ascontiguousarray(
                patches[c * B_CORE : (c + 1) * B_CORE]
                .transpose(1, 0, 2)
                .reshape(KP, B_CORE * NPIX)
            ),
            "w": w_mat,
            "b": b_vec,
        }
        for c in range(N_CORES)
    ]
    nc = _get_program()
    res = run_bass_kernel_spmd(nc, in_maps, list(range(N_CORES)), **spmd_kwargs)
    # y core shard [128, 8192]: partition = t*64+oc, column = G*512+c with
    # G = global bank-col group (g = 2G+t the pixel chunk);
    # pixel-in-core = g*512+c, image = G//4
    y16 = np.stack([r["y"] for r in res.results], axis=0)  # [8, 128, 8192]
    y16 = (
        y16.reshape(N_CORES, 2, OC, B_CORE, 4, 512)  # [core, t, oc, img, G4, c]
        .transpose(0, 3, 2, 4, 1, 5)  # [core, img, oc, G4, t, c]
        .reshape(B_FULL, OC, NPIX)
    )
    # 2*tanh in fp16 then cast to fp32 == cast then *2 (exact: *2 is an
    # exponent bump, in-range for |tanh|<=1)
    out = y16.astype(np.float32).reshape(B_FULL, OC, OH, OW) * np.float32(2.0)
    return out, res


def kernel(x: np.ndarray, weight: np.ndarray, bias: np.ndarray) -> np.ndarray:
    return run_sharded(x, weight, bias)[0]
